# revision 2
# baseline (speedup 1.0000x reference)
"""Trainium2 Bass kernel for BaseModel.forgetting_norm.

Math (per batch b):
    m[t]  = mean over 514 channel*freq rows of x[b, :, t]
    mu[t] = alp[t] * mu[t-1] + (1 - alp[t]) * m[t]          (EMA over time)
    out[b, cf, t] = x[b, cf, t] / (mu[t] + 1e-10)

Mapping (pure data parallel, batch 32 -> 4 per core on 8 cores), bf16:
  - x is cast to bf16 on the host (and the output is upcast back). This
    halves HBM traffic (the kernel is memory-bound: 8.2 MB in + 8.2 MB
    out per core ~ 46 us at 358 GB/s) and runs matmuls at 1 cycle/col
    instead of fp32's 4. Total numerical error stays ~0.5% vs the 2e-2
    tolerance: the channel mean averages 514 independently-rounded
    values so mu is nearly exact; the output pays one bf16 rounding of
    x, of 1/mu, and of the product.
  - channel sums via TensorE with a one-hot [128, 4] lhsT per batch, so
    all 4 batches accumulate into one [4, 2048] PSUM tile (row b = batch
    b) -- no cross-partition copies, and one EMA scan handles all 4
    batches (tensor_tensor_scan runs an independent recurrence per
    partition). The (1-alp)/514 scale is folded into the scan input.
  - the 1e-10 epsilon is dropped: mu ~ 0.5, and 0.5 + 1e-10 rounds to
    0.5 exactly in fp32, so the reference's own add is a no-op.
  - reciprocal of mu computed in a [100, 80] relayout (the divide costs
    8 cycles/element, so spread 4x2000 values over 100 partitions).
  - broadcast of 1/mu across 128 partitions via rank-1 matmul
    (ones[1,128] stationary, bf16 reciprocal row moving), then ScalarE
    copies the PSUM row block to SBUF bf16 so the big multiplies run in
    DVE 2x mode (SBUF bf16 tensor_tensor) instead of 1x from PSUM.
  - all big DMA rides the sync (SP) HWDGE ring in dependency order
    (loads, then the two small relayout transfers, then stores); the
    ACT ring only issues the 8 PSUM->SBUF copies so a store waiting on
    compute can never stall a load.
"""

import sys

sys.path.insert(0, "/opt/trn_rl_repo")

import numpy as np
import ml_dtypes

import concourse.bass as bass
import concourse.bacc as bacc
import concourse.tile as tile
from concourse import mybir
from concourse.bass_utils import run_bass_kernel_spmd

B, C, F, T = 32, 2, 257, 2000
CF = C * F  # 514
NCORES = 8
BL = B // NCORES  # 4 batches per core
NFULL = CF // 128  # 4 full cf blocks
RAG = CF - NFULL * 128  # 2 ragged cf rows

# matmul chunks: 512 fp32 accumulators = one PSUM bank
CHUNKS = [(0, 512), (512, 512), (1024, 512), (1536, 464)]
# halves for the broadcast/multiply/store stage
HALVES = [(0, 1000), (1000, 1000)]
# sub-chunks of one half for the broadcast matmul (bank-aligned)
HCHUNKS = [(0, 512), (512, 488)]
# reciprocal relayout: BL*2000 elems as [100, 80]
PPB, RF = 25, 80


def _build_kernel(nc: bass.Bass, tc: tile.TileContext, ctx):
    f32 = mybir.dt.float32
    bf16 = mybir.dt.bfloat16
    x = nc.dram_tensor("x", [BL, CF, T], bf16, kind="ExternalInput").ap()
    alp4 = nc.dram_tensor("alp4", [BL, T], f32, kind="ExternalInput").ap()
    c14 = nc.dram_tensor("c14", [BL, T], f32, kind="ExternalInput").ap()
    out = nc.dram_tensor("out", [BL, CF, T], bf16, kind="ExternalOutput").ap()

    consts = ctx.enter_context(tc.tile_pool(name="consts", bufs=1))
    xpool = ctx.enter_context(tc.tile_pool(name="xpool", bufs=16))
    ragp = ctx.enter_context(tc.tile_pool(name="ragp", bufs=4))
    rows = ctx.enter_context(tc.tile_pool(name="rows", bufs=1))
    rsbp = ctx.enter_context(tc.tile_pool(name="rsbp", bufs=3))
    mpsum = ctx.enter_context(tc.tile_pool(name="mpsum", bufs=1, space="PSUM"))
    rbcp = ctx.enter_context(tc.tile_pool(name="rbcp", bufs=2, space="PSUM"))

    # one-hot lhsT columns: oh[:, 4b:4b+4] has column b = 1, so
    # lhsT.T @ x adds x's channel-sum into PSUM partition b only.
    oh = consts.tile([128, 4 * BL], bf16)
    nc.vector.memset(oh, 0.0)
    for b in range(BL):
        nc.vector.memset(oh[:, 4 * b + b : 4 * b + b + 1], 1.0)
    ones_row = consts.tile([1, 128], bf16)
    nc.vector.memset(ones_row, 1.0)
    alp_sb = consts.tile([BL, T], f32)
    nc.sync.dma_start(out=alp_sb, in_=alp4)
    c14_sb = consts.tile([BL, T], f32)
    nc.sync.dma_start(out=c14_sb, in_=c14)

    # ---- loads (sync ring) ----
    xt = []
    rag = []
    for b in range(BL):
        tiles_b = []
        for cb in range(NFULL):
            t_ = xpool.tile([128, T], bf16, tag="xt")
            nc.sync.dma_start(out=t_, in_=x[b, cb * 128 : (cb + 1) * 128, :])
            tiles_b.append(t_)
        xt.append(tiles_b)
        r_ = ragp.tile([RAG, T], bf16, tag="rag")
        nc.sync.dma_start(out=r_, in_=x[b, NFULL * 128 :, :])
        rag.append(r_)

    # ---- channel sums for all 4 batches -> m4 [4, 2048] PSUM ----
    # emitted in load-arrival order so the PE FIFO never waits on a DMA
    # that was issued later.
    m4 = mpsum.tile([BL, 2048], f32)
    for b in range(BL):
        for cb in range(NFULL + 1):
            lhsT = (
                oh[:, 4 * b : 4 * b + 4]
                if cb < NFULL
                else oh[0:RAG, 4 * b : 4 * b + 4]
            )
            rhs = xt[b][cb] if cb < NFULL else rag[b]
            for c0, w in CHUNKS:
                nc.tensor.matmul(
                    m4[:, c0 : c0 + w],
                    lhsT,
                    rhs[:, c0 : c0 + w],
                    start=(b == 0 and cb == 0),
                    stop=(b == BL - 1 and cb == NFULL),
                )

    # ---- EMA over time for all 4 batches at once ----
    scanin = rows.tile([BL, T], f32, tag="scanin")
    nc.vector.tensor_mul(scanin, m4[:, 0:T], c14_sb)
    mu4 = rows.tile([BL, T], f32, tag="mu4")
    nc.vector.tensor_tensor_scan(
        mu4, alp_sb, scanin, 0.0, mybir.AluOpType.mult, mybir.AluOpType.add
    )

    # reciprocal in a [100, 80] relayout (8 cyc/elem -> use 100 lanes),
    # emitted on the sync ring after all loads so nothing queues behind it.
    mrel = rows.tile([BL * PPB, RF], f32, tag="mrel")
    nc.sync.dma_start(out=mrel, in_=mu4)
    rrel = rows.tile([BL * PPB, RF], bf16, tag="rrel")
    nc.vector.reciprocal(rrel, mrel)
    # back to one bf16 row: rr_all[0, 2000*b + t] = 1 / mu[b, t]
    rr_all = rows.tile([1, BL * T], bf16, tag="rr_all")
    nc.sync.dma_start(out=rr_all, in_=rrel)

    # ---- per batch: broadcast 1/mu, multiply, store ----
    for b in range(BL):
        for t0, hw in HALVES:
            rbc = rbcp.tile([128, 1024], f32, tag="rbc")
            for s, w in HCHUNKS:
                nc.tensor.matmul(
                    rbc[:, s : s + w],
                    ones_row,
                    rr_all[:, T * b + t0 + s : T * b + t0 + s + w],
                    start=True,
                    stop=True,
                )
            rsb = rsbp.tile([128, 1024], bf16, tag="rsb")
            nc.scalar.copy(out=rsb[:, 0:hw], in_=rbc[:, 0:hw])
            for cb in range(NFULL):
                nc.vector.tensor_mul(
                    xt[b][cb][:, t0 : t0 + hw],
                    xt[b][cb][:, t0 : t0 + hw],
                    rsb[:, 0:hw],
                )
            nc.vector.tensor_mul(
                rag[b][:, t0 : t0 + hw],
                rag[b][:, t0 : t0 + hw],
                rsb[0:RAG, 0:hw],
            )
        for cb in range(NFULL):
            nc.sync.dma_start(
                out=out[b, cb * 128 : (cb + 1) * 128, :], in_=xt[b][cb]
            )
        nc.sync.dma_start(out=out[b, NFULL * 128 :, :], in_=rag[b])


_NC_CACHE = None


def build_bass() -> bass.Bass:
    global _NC_CACHE
    if _NC_CACHE is not None:
        return _NC_CACHE
    import contextlib

    nc = bacc.Bacc("TRN2", debug=False, enable_asserts=True, num_devices=NCORES)
    with tile.TileContext(nc) as tc:
        with contextlib.ExitStack() as ctx:
            _build_kernel(nc, tc, ctx)
    nc.compile()  # reg alloc + event-semaphore wait splitting (1 wait/inst HW limit)
    _NC_CACHE = nc
    return nc


def host_coeffs(sample_length: int):
    """alp[t] exactly as the reference computes it (fp32 ops), plus the
    folded EMA input coefficient (1-alp)/CF."""
    L = int(sample_length)
    alpha = np.float32((L - 1) / (L + 1))
    idx = np.arange(T, dtype=np.float32)
    one = np.float32(1.0)
    alp = np.minimum((idx - one) / (idx + one), alpha).astype(np.float32)
    c14 = ((one - alp) / np.float32(CF)).astype(np.float32)
    alp4 = np.ascontiguousarray(np.broadcast_to(alp, (BL, T)))
    c14_4 = np.ascontiguousarray(np.broadcast_to(c14, (BL, T)))
    return alp4, c14_4


def make_in_maps(input: np.ndarray, sample_length) -> list[dict]:
    """Full f32 input -> per-core input dicts (bf16 x + f32 coeffs)."""
    x = np.asarray(input, dtype=np.float32).reshape(B, CF, T)
    xb = np.ascontiguousarray(x.astype(ml_dtypes.bfloat16))
    alp4, c14_4 = host_coeffs(int(sample_length))
    return [
        {"x": xb[i * BL : (i + 1) * BL], "alp4": alp4, "c14": c14_4}
        for i in range(NCORES)
    ]


def kernel(input: np.ndarray, sample_length) -> np.ndarray:
    in_maps = make_in_maps(input, sample_length)
    nc = build_bass()
    res = run_bass_kernel_spmd(nc, in_maps, core_ids=list(range(NCORES)))
    full = np.concatenate([r["out"] for r in res.results], axis=0)
    return full.astype(np.float32).reshape(B, C, F, T)


if __name__ == "__main__":
    rng = np.random.default_rng(0)
    x = rng.random((B, C, F, T), dtype=np.float32)
    y = kernel(x, 192)
    print(y.shape, y.dtype)
